# revision 18
# baseline (speedup 1.0000x reference)
"""Causal self-attention (B=4, T=2048, C=1024, H=16) on 8 trn2 NeuronCores.

Sharding: core = (batch b, head-group g) with b = core//2, g = core%2.
Each core handles one batch and 8 heads (column-parallel qkv, row-parallel
out_proj).  Cores return partial out-projection results; the host sums the
two head-group partials per batch and adds b_out (the "all-reduce" of the
row-parallel out_proj done host-side during unshard).

Per-core kernel (all matmuls bf16 with fp32 PSUM accumulate):
  - qk projection into d-major layout qkT [1024, T] (= W^T @ x^T), bias added
    per-partition at PSUM eviction.
  - v  projection into T-major layout V [T, 512], bias folded in via a K=1
    matmul with a ones stationary vector.
  - scores computed transposed: S^T[k, q] = kT_tile^T @ qT (contraction d=64;
    even/odd heads of a 128-partition block sit at base partitions 0/64 so
    the PE can row-tile them concurrently).
  - p = exp(0.125 * S^T) on the scalar engine (no max subtraction; logits are
    bounded for this data), causal masking by multiplying diagonal-straddling
    tiles with precomputed 0/1 masks on the vector engine.
  - y^T (+ softmax denominators) via one fused matmul with lhsT = [V | 1].
  - normalization: reciprocal on DVE, partition-broadcast on GPSIMD, multiply
    on DVE, small SBUF->SBUF DMA to place odd heads at partitions 64..127.
  - out projection: out[t, c] accumulated over 4 hd-blocks of y^T @ w_out.
"""

import os
import numpy as np

try:
    import concourse.bass as bass
except ImportError:  # pragma: no cover
    import sys

    sys.path.insert(0, "/opt/trn_rl_repo")
    import concourse.bass as bass

import concourse.mybir as mybir
from concourse.bacc import Bacc
from concourse.bass_utils import run_bass_kernel_spmd
from concourse.tile import TileContext
from ml_dtypes import bfloat16

B, T, C, H = 4, 2048, 1024, 16
HD = 64        # head dim
G = 512        # head-group width: 8 heads * 64
P = 128
NCT = C // P   # contraction tiles over C
NTT = T // P   # 128-row tiles over T
CHW = 512      # q-chunk width
NCH = T // CHW
SPAN = 3       # k-tiles per S^T span (one PSUM tile = 3 banks)

F32 = mybir.dt.float32
BF16 = mybir.dt.bfloat16

_BUILT = None
LAST_EXEC_TIME_NS = None


def _build_bass():
    nc = Bacc("TRN2", target_bir_lowering=False)

    xt_d = nc.dram_tensor("xt", [C, T], BF16, kind="ExternalInput")
    wqk_d = nc.dram_tensor("wqk", [C, 2 * G], BF16, kind="ExternalInput")
    bqk_d = nc.dram_tensor("bqk", [P, 8], F32, kind="ExternalInput")
    wv_d = nc.dram_tensor("wv", [C, G], BF16, kind="ExternalInput")
    bv_d = nc.dram_tensor("bv", [1, G], BF16, kind="ExternalInput")
    wo_d = nc.dram_tensor("wo", [P, 4, C], BF16, kind="ExternalInput")
    mask_d = nc.dram_tensor("mask", [P, 4, CHW], BF16, kind="ExternalInput")
    out_d = nc.dram_tensor("out", [T, C], F32, kind="ExternalOutput")

    Exp = mybir.ActivationFunctionType.Exp

    with TileContext(nc) as tc:
        with (
            tc.tile_pool(name="const", bufs=1) as cpool,
            tc.tile_pool(name="work", bufs=3) as wpool,
            tc.tile_pool(name="ppool", bufs=4) as ppool,
            tc.tile_pool(name="pspool", bufs=2, space="PSUM") as pspool,
            tc.tile_pool(name="avpool", bufs=2, space="PSUM") as avpool,
        ):
            xts = cpool.tile([P, NCT, T], BF16, tag="xts")
            wqks = cpool.tile([P, NCT, 2 * G], BF16, tag="wqks")
            bqks = cpool.tile([P, 8], F32, tag="bqks")
            wvs = cpool.tile([P, NCT, G], BF16, tag="wvs")
            bvs = cpool.tile([1, G], BF16, tag="bvs")
            wos = cpool.tile([P, 4, C], BF16, tag="wos")
            masks = cpool.tile([P, 4, CHW], BF16, tag="masks")
            ones = cpool.tile([1, P], BF16, tag="ones")
            ones64 = cpool.tile([P, HD], F32, tag="ones64")  # row 64 = 1.0
            qkts = cpool.tile([P, 8, T], BF16, tag="qkts")
            vs = cpool.tile([P, NTT, 8, HD + 1], BF16, tag="vs")
            yts = cpool.tile([P, 4, T], BF16, tag="yts")

            nc.gpsimd.dma_start(out=xts[:, :, :], in_=xt_d.rearrange("(a p) t -> p a t", p=P))
            nc.gpsimd.dma_start(out=wqks[:, :, :], in_=wqk_d.rearrange("(a p) m -> p a m", p=P))
            nc.gpsimd.dma_start(out=bqks[:, :], in_=bqk_d[:, :])
            nc.gpsimd.dma_start(out=wvs[:, :, :], in_=wv_d.rearrange("(a p) m -> p a m", p=P))
            nc.gpsimd.dma_start(out=bvs[:, :], in_=bv_d[:, :])
            nc.gpsimd.dma_start(out=wos[:, :, :], in_=wo_d[:, :, :])
            nc.gpsimd.dma_start(out=masks[:, :, :], in_=mask_d[:, :, :])
            nc.vector.memset(ones[:, :], 1.0)
            nc.vector.memset(ones64[HD:HD + 1, :], 1.0)

            # ---- V phase: V[t, (h d)] = x @ wv + bv, T-major ----
            for tt in range(NTT):
                ps = pspool.tile([P, 3 * CHW], F32, tag="s")
                pv = ps[:, 0:G]
                for ct in range(NCT):
                    nc.tensor.matmul(
                        pv,
                        lhsT=xts[:, ct, tt * P:(tt + 1) * P],
                        rhs=wvs[:, ct, :],
                        start=(ct == 0),
                        stop=False,
                    )
                nc.tensor.matmul(pv, lhsT=ones[:, :], rhs=bvs[:, :], start=False, stop=True)
                nc.vector.memset(vs[:, tt, :, HD:HD + 1], 1.0)
                nc.vector.tensor_copy(
                    out=vs[:, tt, :, 0:HD],
                    in_=pv.rearrange("p (h d) -> p h d", h=8),
                )

            # ---- per head-pair: qk projection, then attention ----
            for hp in range(4):
                for mt in (hp, 4 + hp):  # q block then k block
                    for ch in range(NCH):
                        ps = pspool.tile([P, 3 * CHW], F32, tag="s")
                        pq = ps[:, 0:CHW]
                        for ct in range(NCT):
                            nc.tensor.matmul(
                                pq,
                                lhsT=wqks[:, ct, mt * P:(mt + 1) * P],
                                rhs=xts[:, ct, ch * CHW:(ch + 1) * CHW],
                                start=(ct == 0),
                                stop=(ct == NCT - 1),
                            )
                        nc.vector.tensor_scalar_add(
                            out=qkts[:, mt, ch * CHW:(ch + 1) * CHW],
                            in0=pq,
                            scalar1=bqks[:, mt:mt + 1],
                        )

                for ch in range(NCH):
                    nkt = 4 * ch + 4
                    avs = {}
                    for par in (0, 1):
                        avs[par] = avpool.tile([P, CHW], F32, tag="av", name=f"av_{hp}_{ch}_{par}")
                    for s0 in range(0, nkt, SPAN):
                        ns = min(SPAN, nkt - s0)
                        sps = {}
                        pts = {}
                        for par in (0, 1):
                            po = 64 * par
                            sp = pspool.tile([P, 3 * CHW], F32, tag="s", name=f"sp_{hp}_{ch}_{s0}_{par}")
                            for i in range(ns):
                                kt = s0 + i
                                nc.tensor.matmul(
                                    sp[:, i * CHW:(i + 1) * CHW],
                                    lhsT=qkts[po:po + 64, 4 + hp, kt * P:(kt + 1) * P],
                                    rhs=qkts[po:po + 64, hp, ch * CHW:(ch + 1) * CHW],
                                    start=True,
                                    stop=True,
                                )
                            sps[par] = sp
                        for par in (0, 1):
                            pt = ppool.tile([P, 3 * CHW], BF16, tag="p", name=f"pt_{hp}_{ch}_{s0}_{par}")
                            nc.scalar.activation(
                                out=pt[:, 0:ns * CHW],
                                in_=sps[par][:, 0:ns * CHW],
                                func=Exp,
                                scale=0.125,
                            )
                            for i in range(ns):
                                kt = s0 + i
                                if kt >= 4 * ch:
                                    r = kt - 4 * ch
                                    nc.vector.tensor_mul(
                                        out=pt[:, i * CHW:(i + 1) * CHW],
                                        in0=pt[:, i * CHW:(i + 1) * CHW],
                                        in1=masks[:, r, :],
                                    )
                            pts[par] = pt
                        for par in (0, 1):
                            hl = 2 * hp + par
                            for i in range(ns):
                                kt = s0 + i
                                nc.tensor.matmul(
                                    avs[par][0:HD + 1, :],
                                    lhsT=vs[:, kt, hl, :],
                                    rhs=pts[par][:, i * CHW:(i + 1) * CHW],
                                    start=(kt == 0),
                                    stop=(kt == nkt - 1),
                                )
                    # normalize y^T by the softmax denominator (row HD of av)
                    for par in (0, 1):
                        yun = wpool.tile([P, CHW], F32, tag="yun")
                        nc.vector.tensor_copy(out=yun[0:HD + 1, :], in_=avs[par][0:HD + 1, :])
                        nc.vector.reciprocal(out=yun[HD:HD + 1, :], in_=yun[HD:HD + 1, :])
                        # broadcast the reciprocal row across 64 partitions with
                        # a K=1 matmul (ones column at base partition 64 to
                        # match the rhs base).
                        recb = avpool.tile([P, CHW], F32, tag="av", name=f"recb_{hp}_{ch}_{par}")
                        nc.tensor.matmul(
                            recb[0:HD, :],
                            lhsT=ones64[HD:HD + 1, :],
                            rhs=yun[HD:HD + 1, :],
                            start=True,
                            stop=True,
                        )
                        ynorm = wpool.tile([HD, CHW], BF16, tag="ynorm")
                        nc.vector.tensor_mul(out=ynorm[:, :], in0=yun[0:HD, :], in1=recb[0:HD, :])
                        nc.gpsimd.dma_start(
                            out=yts[64 * par:64 * par + 64, hp, ch * CHW:(ch + 1) * CHW],
                            in_=ynorm[:, :],
                        )

            # ---- out projection: out[t, c] = sum_j yts[:, j] ^T @ wo[:, j] ----
            for tt in range(NTT):
                for cc in range(2):
                    ps = pspool.tile([P, 3 * CHW], F32, tag="s")
                    po = ps[:, 0:CHW]
                    for j in range(4):
                        nc.tensor.matmul(
                            po,
                            lhsT=yts[:, j, tt * P:(tt + 1) * P],
                            rhs=wos[:, j, cc * CHW:(cc + 1) * CHW],
                            start=(j == 0),
                            stop=(j == 3),
                        )
                    ev = wpool.tile([P, CHW], F32, tag="ev")
                    nc.vector.tensor_copy(out=ev[:, :], in_=po)
                    nc.sync.dma_start(
                        out=out_d[tt * P:(tt + 1) * P, cc * CHW:(cc + 1) * CHW],
                        in_=ev[:, :],
                    )

    nc.finalize()
    return nc


def _make_masks():
    p = np.arange(P)[:, None]
    f = np.arange(CHW)[None, :]
    m = np.empty((P, 4, CHW), dtype=bfloat16)
    for r in range(4):
        m[:, r, :] = ((P * r + p) <= f).astype(bfloat16)
    return m


def _core_inputs(x, w_qkv, b_qkv, w_out, core, masks):
    b, g = core // 2, core % 2
    qc = slice(G * g, G * g + G)
    kc = slice(C + G * g, C + G * g + G)
    vc = slice(2 * C + G * g, 2 * C + G * g + G)
    xt = np.ascontiguousarray(x[b].T).astype(bfloat16)
    wqk = np.ascontiguousarray(
        np.concatenate([w_qkv[:, qc], w_qkv[:, kc]], axis=1)
    ).astype(bfloat16)
    bqk = np.ascontiguousarray(
        np.concatenate([b_qkv[qc], b_qkv[kc]]).reshape(8, P).T
    ).astype(np.float32)
    wv = np.ascontiguousarray(w_qkv[:, vc]).astype(bfloat16)
    bv = np.ascontiguousarray(b_qkv[vc].reshape(1, G)).astype(bfloat16)
    wo = np.ascontiguousarray(
        w_out[G * g:G * g + G, :].reshape(4, P, C).transpose(1, 0, 2)
    ).astype(bfloat16)
    return {
        "xt": xt,
        "wqk": wqk,
        "bqk": bqk,
        "wv": wv,
        "bv": bv,
        "wo": wo,
        "mask": masks,
    }


def kernel(x, w_qkv, b_qkv, w_out, b_out):
    global _BUILT, LAST_EXEC_TIME_NS
    x = np.asarray(x, dtype=np.float32)
    w_qkv = np.asarray(w_qkv, dtype=np.float32)
    b_qkv = np.asarray(b_qkv, dtype=np.float32)
    w_out = np.asarray(w_out, dtype=np.float32)
    b_out = np.asarray(b_out, dtype=np.float32)

    if _BUILT is None:
        _BUILT = _build_bass()
    nc = _BUILT

    masks = _make_masks()
    in_maps = [
        _core_inputs(x, w_qkv, b_qkv, w_out, core, masks) for core in range(8)
    ]
    trace = bool(int(os.environ.get("KERNEL_TRACE", "0")))
    res = run_bass_kernel_spmd(nc, in_maps, list(range(8)), trace=trace)
    LAST_EXEC_TIME_NS = res.exec_time_ns

    out = np.empty((B, T, C), dtype=np.float32)
    for b in range(B):
        out[b] = res.results[2 * b]["out"] + res.results[2 * b + 1]["out"] + b_out
    return out
